# revision 7
# baseline (speedup 1.0000x reference)
# HEPOS cross-attention (strided per-head K/V) on 8 Trainium2 NeuronCores.
#
# Reference computation (per head h, stride s = STRIDE_LIST[h]):
#   Q = x @ Wq.T + bq ; K = e @ Wk.T + bk ; V = e @ Wv.T + bv
#   out_h = softmax(Q_h @ K_h[::s].T / 8) @ V_h[::s]
#   out   = concat_h(out_h) @ Wo.T + bo
#
# Sharding: 64 (batch, head) units over 8 cores. Core c owns head group
# g = c % 4 (heads 4g..4g+3, strides [1,2,4,8] -- one of each stride, so
# per-core work is identical) and batch pair [0,1] (c < 4) or [2,3]
# (c >= 4). Each core computes its heads' contribution to out (partial
# out = concat @ Wo cols) for its two batches; the host sums the four
# partials per batch and adds bo.
#
# On-device layout convention: activations live transposed (D on the
# SBUF partition dim), prepared by the host with numpy. The stride is
# folded into the K/V projections (only strided encoder rows are
# projected). Scores are computed transposed ([S_chunk, T]) so the
# attention matmul needs no transposes anywhere; the softmax denominator
# falls out of a ones-column appended to the V stationary operand.

import os
import sys

import numpy as np

for _p in ("/opt/trn_rl_repo", "/root/.axon_site/_ro/trn_rl_repo"):
    if os.path.isdir(_p) and _p not in sys.path:
        sys.path.insert(0, _p)

import concourse.bass as bass  # noqa: E402
import concourse.tile as tile  # noqa: E402
from concourse import bacc, mybir  # noqa: E402
from concourse import bass_utils  # noqa: E402

F32 = mybir.dt.float32
MM_DT = mybir.dt.float32r  # full-rate fp32 path on the PE array
AF = mybir.ActivationFunctionType

D_MODEL = 1024
NUM_HEADS = 16
HEAD_DIM = 64
STRIDE_LIST = [1, 2, 4, 8] * 4
B, T, S = 4, 1024, 4096
N_CORES = 8

FULL_CFG = dict(
    nb=2,  # batches per core
    T=T,
    S=S,
    D=D_MODEL,
    nh=4,  # heads per core
    strides=(1, 2, 4, 8),
    hd=HEAD_DIM,
    blk=512,  # encoder S-block columns processed per iteration
    tt=512,  # T tile (PSUM free-dim limit for fp32)
)


FR = MM_DT  # fp32r: all matmul operands are produced/stored in this dtype


def _mm(nc, out, lhsT, rhs, start, stop):
    nc.tensor.matmul(out, lhsT, rhs, start=start, stop=stop)


def build_program(cfg):
    """Build the per-core Bass/Tile program (same program on all cores)."""
    nb, Tl, Sl, Dl = cfg["nb"], cfg["T"], cfg["S"], cfg["D"]
    nh, strides, hd = cfg["nh"], cfg["strides"], cfg["hd"]
    blk, tt = cfg["blk"], cfg["tt"]
    ndc = Dl // 128  # contraction chunks
    nblk = Sl // blk
    ntt = Tl // tt
    nhp = (nh * hd + 127) // 128  # 128-row passes over the packed heads
    HP = nh * hd  # packed head rows (e.g. 256)

    nc = bacc.Bacc(
        "TRN2",
        target_bir_lowering=False,
        debug=False,
        enable_asserts=False,
        num_devices=N_CORES,
    )

    xT = nc.dram_tensor("xT", [Dl, nb * Tl], MM_DT, kind="ExternalInput").ap()
    eT = nc.dram_tensor("eT", [Dl, nb * Sl], MM_DT, kind="ExternalInput").ap()
    wqT = nc.dram_tensor("wqT", [Dl, HP], MM_DT, kind="ExternalInput").ap()
    wkT = nc.dram_tensor("wkT", [Dl, HP], MM_DT, kind="ExternalInput").ap()
    wvT = nc.dram_tensor("wvT", [Dl, HP], MM_DT, kind="ExternalInput").ap()
    woT = nc.dram_tensor("woT", [HP, Dl], MM_DT, kind="ExternalInput").ap()
    bqd = nc.dram_tensor("bq", [nh, hd, 1], F32, kind="ExternalInput").ap()
    bkd = nc.dram_tensor("bk", [nh, hd, 1], F32, kind="ExternalInput").ap()
    bvd = nc.dram_tensor("bvb", [128, HP], F32, kind="ExternalInput").ap()
    out = nc.dram_tensor("partial", [nb * Tl, Dl], F32, kind="ExternalOutput").ap()

    with tile.TileContext(nc) as tc:
        _build_tile(tc, cfg, xT, eT, wqT, wkT, wvT, woT, bqd, bkd, bvd, out)

    nc.compile()
    return nc


def _build_tile(tc, cfg, xT, eT, wqT, wkT, wvT, woT, bqd, bkd, bvd, out):
    nc = tc.nc
    nb, Tl, Sl, Dl = cfg["nb"], cfg["T"], cfg["S"], cfg["D"]
    nh, strides, hd = cfg["nh"], cfg["strides"], cfg["hd"]
    blk, tt = cfg["blk"], cfg["tt"]
    ndc = Dl // 128
    nblk = Sl // blk
    ntt = Tl // tt
    HP = nh * hd
    nhp = (HP + 127) // 128

    from contextlib import ExitStack

    with ExitStack() as ctx:
        wpool = ctx.enter_context(tc.tile_pool(name="weights", bufs=1))
        qtpool = ctx.enter_context(tc.tile_pool(name="qt", bufs=1))
        etpool = ctx.enter_context(tc.tile_pool(name="et", bufs=2))
        ktpool = ctx.enter_context(tc.tile_pool(name="kt", bufs=3))
        vpool = ctx.enter_context(tc.tile_pool(name="v", bufs=8))
        ppool = ctx.enter_context(tc.tile_pool(name="p", bufs=3))
        avpool = ctx.enter_context(tc.tile_pool(name="avacc", bufs=1))
        opool = ctx.enter_context(tc.tile_pool(name="outs", bufs=3))
        npool = ctx.enter_context(tc.tile_pool(name="norm", bufs=1))
        qo_ps = ctx.enter_context(tc.tile_pool(name="qo_ps", bufs=2, space="PSUM"))
        sc_ps = ctx.enter_context(tc.tile_pool(name="sc_ps", bufs=2, space="PSUM"))
        kv_ps = ctx.enter_context(tc.tile_pool(name="kv_ps", bufs=2, space="PSUM"))
        av_ps = ctx.enter_context(tc.tile_pool(name="av_ps", bufs=2, space="PSUM"))

        # ---- weights into SBUF ----
        wq_sb = [wpool.tile([128, HP], FR, tag=f"wq{dc}", name="wq_sb") for dc in range(ndc)]
        wk_sb = [wpool.tile([128, HP], FR, tag=f"wk{dc}", name="wk_sb") for dc in range(ndc)]
        wv_sb = [wpool.tile([128, HP], FR, tag=f"wv{dc}", name="wv_sb") for dc in range(ndc)]
        for dc in range(ndc):
            nc.sync.dma_start(out=wq_sb[dc], in_=wqT[dc * 128 : (dc + 1) * 128, :])
            nc.sync.dma_start(out=wk_sb[dc], in_=wkT[dc * 128 : (dc + 1) * 128, :])
            nc.sync.dma_start(out=wv_sb[dc], in_=wvT[dc * 128 : (dc + 1) * 128, :])
        wo_sb = [wpool.tile([hd, Dl], FR, tag=f"wo{h}", name="wo_sb") for h in range(nh)]
        for h in range(nh):
            nc.sync.dma_start(out=wo_sb[h], in_=woT[h * hd : (h + 1) * hd, :])
        bq_sb = wpool.tile([hd, nh], F32, tag="bq", name="bq_sb")
        bk_sb = wpool.tile([hd, nh], F32, tag="bk", name="bk_sb")
        for h in range(nh):
            nc.sync.dma_start(out=bq_sb[:, h : h + 1], in_=bqd[h])
            nc.sync.dma_start(out=bk_sb[:, h : h + 1], in_=bkd[h])
        bv_sb = wpool.tile([128, HP], F32, tag="bv", name="bv_sb")
        nc.sync.dma_start(out=bv_sb, in_=bvd)
        ones_sb = wpool.tile([128, 1], F32, tag="ones", name="ones_sb")
        nc.vector.memset(ones_sb, 1.0)

        # ---- phase 1: Q^T = (x @ Wq.T + bq)^T, packed heads on partitions ----
        qt_sb = {}  # (b, pass) -> [128, T] tile
        with tc.tile_pool(name="xt", bufs=1) as xpool:
            for b in range(nb):
                xts = []
                for dc in range(ndc):
                    xt = xpool.tile([128, Tl], FR, tag=f"xt{dc}", name="xt")
                    nc.sync.dma_start(
                        out=xt,
                        in_=xT[dc * 128 : (dc + 1) * 128, b * Tl : (b + 1) * Tl],
                    )
                    xts.append(xt)
                for h in range(nh):
                    qt = qtpool.tile([hd, Tl], FR, tag=f"qt{b}{h}", name="qt")
                    qt_sb[(b, h)] = qt
                    for nt in range(ntt):
                        ps = qo_ps.tile([128, tt], F32, tag="qo", name="q_psum")
                        for dc in range(ndc):
                            _mm(
                                nc,
                                ps[:hd, :],
                                wq_sb[dc][:, h * hd : (h + 1) * hd],
                                xts[dc][:, nt * tt : (nt + 1) * tt],
                                start=(dc == 0),
                                stop=(dc == ndc - 1),
                            )
                        nc.scalar.activation(
                            qt[:, nt * tt : (nt + 1) * tt],
                            ps[:hd, :],
                            AF.Identity,
                            bias=bq_sb[:, h : h + 1],
                        )

        # ---- phase 2+3 per batch ----
        for b in range(nb):
            avacc = {}
            for h in range(nh):
                avacc[h] = avpool.tile([hd + 1, Tl], F32, tag=f"av{h}", name="avacc")
            for ib in range(nblk):
                et = []
                for dc in range(ndc):
                    t = etpool.tile([128, blk], FR, tag=f"et{dc}", name="et_t")
                    nc.sync.dma_start(
                        out=t,
                        in_=eT[
                            dc * 128 : (dc + 1) * 128,
                            b * Sl + ib * blk : b * Sl + (ib + 1) * blk,
                        ],
                    )
                    et.append(t)
                for h in range(nh):
                    s = strides[h]
                    ncol = blk // s  # strided K/V rows in this block
                    # K^T_h for this block: [hd, ncol]
                    kt = ktpool.tile([hd, blk], FR, name="kt")
                    for c0 in range(0, ncol, tt):
                        cw = min(tt, ncol - c0)
                        kps = kv_ps.tile([128, tt], F32, tag="kv", name="kv_psum")
                        for dc in range(ndc):
                            _mm(
                                nc,
                                kps[:hd, :cw],
                                wk_sb[dc][:, h * hd : (h + 1) * hd],
                                et[dc][:, c0 * s : (c0 + cw) * s : s],
                                start=(dc == 0),
                                stop=(dc == ndc - 1),
                            )
                        nc.scalar.activation(
                            kt[:, c0 : c0 + cw],
                            kps[:hd, :cw],
                            AF.Identity,
                            bias=bk_sb[:, h : h + 1],
                        )
                    # V chunks + scores + exp + AV accumulation
                    nck = (ncol + 127) // 128
                    avp = [
                        av_ps.tile([hd + 1, tt], F32, tag="av_ps", name="av_psum") for _ in range(ntt)
                    ]
                    for ck in range(nck):
                        rw = min(128, ncol - ck * 128)
                        vt = vpool.tile([128, hd + 8], FR, tag="v", name="vt")
                        vps = kv_ps.tile([128, tt], F32, tag="kv", name="kv_psum")
                        for dc in range(ndc):
                            _mm(
                                nc,
                                vps[:rw, :hd],
                                et[dc][:, ck * 128 * s : (ck * 128 + rw) * s : s],
                                wv_sb[dc][:, h * hd : (h + 1) * hd],
                                start=(dc == 0),
                                stop=(dc == ndc - 1),
                            )
                        nc.vector.tensor_add(
                            vt[:rw, :hd], vps[:rw, :hd], bv_sb[:rw, h * hd : (h + 1) * hd]
                        )
                        nc.vector.tensor_copy(vt[:rw, hd : hd + 1], ones_sb[:rw])
                        # scores^T chunk: [rw, T], then P = exp(scores/8)
                        pt = ppool.tile([128, Tl], FR, tag="p", name="pt")
                        for nt in range(ntt):
                            sps = sc_ps.tile([128, tt], F32, tag="sc", name="sc_psum")
                            _mm(
                                nc,
                                sps[:rw, :],
                                kt[:, ck * 128 : ck * 128 + rw],
                                qt_sb[(b, h)][:, nt * tt : (nt + 1) * tt],
                                start=True,
                                stop=True,
                            )
                            nc.scalar.activation(
                                pt[:rw, nt * tt : (nt + 1) * tt],
                                sps[:rw, :],
                                AF.Exp,
                                scale=1.0 / float(np.sqrt(hd)),
                            )
                            _mm(
                                nc,
                                avp[nt],
                                vt[:rw, : hd + 1],
                                pt[:rw, nt * tt : (nt + 1) * tt],
                                start=(ck == 0),
                                stop=(ck == nck - 1),
                            )
                    for nt in range(ntt):
                        dst = avacc[h][:, nt * tt : (nt + 1) * tt]
                        if ib == 0:
                            nc.vector.tensor_copy(dst, avp[nt])
                        else:
                            nc.vector.tensor_add(dst, dst, avp[nt])

            # ---- phase 3: normalize + output projection ----
            ot_sb = {}
            for h in range(nh):
                r = npool.tile([1, Tl], F32, tag="recip", name="recip")
                nc.vector.reciprocal(r, avacc[h][hd : hd + 1, :])
                rb = npool.tile([hd, Tl], F32, tag="rb", name="rbcast")
                nc.gpsimd.partition_broadcast(rb, r)
                ot = npool.tile([hd, Tl], FR, tag=f"ot{h}", name="ot")
                nc.vector.tensor_mul(ot, avacc[h][:hd, :], rb)
                ot_sb[h] = ot
            for tc_i in range(Tl // 128):
                for nt in range(0, Dl, tt):
                    ops = qo_ps.tile([128, tt], F32, tag="qo", name="q_psum")
                    for h in range(nh):
                        _mm(
                            nc,
                            ops,
                            ot_sb[h][:, tc_i * 128 : (tc_i + 1) * 128],
                            wo_sb[h][:, nt : nt + tt],
                            start=(h == 0),
                            stop=(h == nh - 1),
                        )
                    ob = opool.tile([128, tt], F32, tag="ob", name="ob")
                    nc.vector.tensor_copy(ob, ops)
                    nc.sync.dma_start(
                        out=out[
                            b * Tl + tc_i * 128 : b * Tl + (tc_i + 1) * 128,
                            nt : nt + tt,
                        ],
                        in_=ob,
                    )


# ---------------------------------------------------------------------------
# Host-side sharding / gathering
# ---------------------------------------------------------------------------


def _core_map():
    """core -> (batches, heads)"""
    m = {}
    for c in range(N_CORES):
        g = c % 4
        bs = [0, 1] if c < 4 else [2, 3]
        hs = [4 * g + i for i in range(4)]
        m[c] = (bs, hs)
    return m


def shard_inputs(inputs, cfg):
    x = np.asarray(inputs["decoder_input"], np.float32)
    e = np.asarray(inputs["encoder_output"], np.float32)
    Wq = np.asarray(inputs["Wq"], np.float32)
    Wk = np.asarray(inputs["Wk"], np.float32)
    Wv = np.asarray(inputs["Wv"], np.float32)
    Wo = np.asarray(inputs["Wo"], np.float32)
    bq = np.asarray(inputs["bq"], np.float32)
    bk = np.asarray(inputs["bk"], np.float32)
    bv = np.asarray(inputs["bv"], np.float32)
    hd = cfg["hd"]
    nh = cfg["nh"]
    in_maps = []
    for c, (bs, hs) in _core_map().items():
        rows = np.concatenate([np.arange(h * hd, (h + 1) * hd) for h in hs])
        xT = np.ascontiguousarray(
            x[bs].reshape(len(bs) * cfg["T"], cfg["D"]).T
        )
        eTc = np.ascontiguousarray(
            e[bs].reshape(len(bs) * cfg["S"], cfg["D"]).T
        )
        in_maps.append(
            {
                "xT": xT,
                "eT": eTc,
                "wqT": np.ascontiguousarray(Wq[rows].T),
                "wkT": np.ascontiguousarray(Wk[rows].T),
                "wvT": np.ascontiguousarray(Wv[rows].T),
                "woT": np.ascontiguousarray(Wo[:, rows].T),
                "bq": np.ascontiguousarray(bq[rows].reshape(nh, hd, 1)),
                "bk": np.ascontiguousarray(bk[rows].reshape(nh, hd, 1)),
                "bvb": np.ascontiguousarray(
                    np.tile(bv[rows][None, :], (128, 1))
                ),
            }
        )
    return in_maps


def gather_output(results, bo, cfg):
    Tl, Dl = cfg["T"], cfg["D"]
    out = np.zeros((B, Tl, Dl), np.float32)
    for c, (bs, _hs) in _core_map().items():
        p = results[c]["partial"].reshape(len(bs), Tl, Dl)
        for i, b in enumerate(bs):
            out[b] += p[i]
    return out + np.asarray(bo, np.float32)[None, None, :]


_COMPILED = None


def _get_compiled():
    global _COMPILED
    if _COMPILED is None:
        _COMPILED = build_program(FULL_CFG)
    return _COMPILED


def run_on_cores(inputs, trace=False, **kw):
    nc = _get_compiled()
    in_maps = shard_inputs(inputs, FULL_CFG)
    res = bass_utils.run_bass_kernel_spmd(
        nc, in_maps, core_ids=list(range(N_CORES)), trace=trace, **kw
    )
    return res


def kernel(**inputs) -> np.ndarray:
    res = run_on_cores(inputs, trace=False)
    return gather_output(res.results, inputs["bo"], FULL_CFG)


# revision 8
# speedup vs baseline: 1.2466x; 1.2466x over previous
# HEPOS cross-attention (strided per-head K/V) on 8 Trainium2 NeuronCores.
#
# Reference computation (per head h, stride s = STRIDE_LIST[h]):
#   Q = x @ Wq.T + bq ; K = e @ Wk.T + bk ; V = e @ Wv.T + bv
#   out_h = softmax(Q_h @ K_h[::s].T / 8) @ V_h[::s]
#   out   = concat_h(out_h) @ Wo.T + bo
#
# Sharding: 64 (batch, head) units over 8 cores. Core c owns head group
# g = c % 4 (heads 4g..4g+3, strides [1,2,4,8] -- one of each stride, so
# per-core work is identical) and batch pair [0,1] (c < 4) or [2,3]
# (c >= 4). Each core computes its heads' contribution to out (partial
# out = concat @ Wo cols) for its two batches; the host sums the four
# partials per batch and adds bo.
#
# On-device layout convention: activations live transposed (D on the
# SBUF partition dim), prepared by the host with numpy. The stride is
# folded into the K/V projections (only strided encoder rows are
# projected). Scores are computed transposed ([S_chunk, T]) so the
# attention matmul needs no transposes anywhere; the softmax denominator
# falls out of a ones-column appended to the V stationary operand.

import os
import sys

import ml_dtypes
import numpy as np

BF16 = ml_dtypes.bfloat16

for _p in ("/opt/trn_rl_repo", "/root/.axon_site/_ro/trn_rl_repo"):
    if os.path.isdir(_p) and _p not in sys.path:
        sys.path.insert(0, _p)

import concourse.bass as bass  # noqa: E402
import concourse.tile as tile  # noqa: E402
from concourse import bacc, mybir  # noqa: E402
from concourse import bass_utils  # noqa: E402

F32 = mybir.dt.float32
MM_DT = mybir.dt.bfloat16  # matmul operand dtype: full PE rate, half DMA
AF = mybir.ActivationFunctionType

D_MODEL = 1024
NUM_HEADS = 16
HEAD_DIM = 64
STRIDE_LIST = [1, 2, 4, 8] * 4
B, T, S = 4, 1024, 4096
N_CORES = 8

FULL_CFG = dict(
    nb=2,  # batches per core
    T=T,
    S=S,
    D=D_MODEL,
    nh=4,  # heads per core
    strides=(1, 2, 4, 8),
    hd=HEAD_DIM,
    blk=512,  # encoder S-block columns processed per iteration
    tt=512,  # T tile (PSUM free-dim limit for fp32)
)


FR = MM_DT  # fp32r: all matmul operands are produced/stored in this dtype


def _mm(nc, out, lhsT, rhs, start, stop):
    nc.tensor.matmul(out, lhsT, rhs, start=start, stop=stop)


def build_program(cfg):
    """Build the per-core Bass/Tile program (same program on all cores)."""
    nb, Tl, Sl, Dl = cfg["nb"], cfg["T"], cfg["S"], cfg["D"]
    nh, strides, hd = cfg["nh"], cfg["strides"], cfg["hd"]
    blk, tt = cfg["blk"], cfg["tt"]
    ndc = Dl // 128  # contraction chunks
    nblk = Sl // blk
    ntt = Tl // tt
    nhp = (nh * hd + 127) // 128  # 128-row passes over the packed heads
    HP = nh * hd  # packed head rows (e.g. 256)

    nc = bacc.Bacc(
        "TRN2",
        target_bir_lowering=False,
        debug=False,
        enable_asserts=False,
        num_devices=N_CORES,
    )

    xT = nc.dram_tensor("xT", [Dl, nb * Tl], MM_DT, kind="ExternalInput").ap()
    eT = nc.dram_tensor("eT", [Dl, nb * Sl], MM_DT, kind="ExternalInput").ap()
    wqT = nc.dram_tensor("wqT", [Dl, HP], MM_DT, kind="ExternalInput").ap()
    wkT = nc.dram_tensor("wkT", [Dl, HP], MM_DT, kind="ExternalInput").ap()
    wvT = nc.dram_tensor("wvT", [Dl, HP], MM_DT, kind="ExternalInput").ap()
    woT = nc.dram_tensor("woT", [HP, Dl], MM_DT, kind="ExternalInput").ap()
    bqd = nc.dram_tensor("bq", [nh, hd, 1], F32, kind="ExternalInput").ap()
    bkd = nc.dram_tensor("bk", [nh, hd, 1], F32, kind="ExternalInput").ap()
    bvd = nc.dram_tensor("bvb", [128, HP], F32, kind="ExternalInput").ap()
    out = nc.dram_tensor("partial", [nb * Tl, Dl], F32, kind="ExternalOutput").ap()

    with tile.TileContext(nc) as tc:
        _build_tile(tc, cfg, xT, eT, wqT, wkT, wvT, woT, bqd, bkd, bvd, out)

    nc.compile()
    return nc


def _build_tile(tc, cfg, xT, eT, wqT, wkT, wvT, woT, bqd, bkd, bvd, out):
    nc = tc.nc
    nb, Tl, Sl, Dl = cfg["nb"], cfg["T"], cfg["S"], cfg["D"]
    nh, strides, hd = cfg["nh"], cfg["strides"], cfg["hd"]
    blk, tt = cfg["blk"], cfg["tt"]
    ndc = Dl // 128
    nblk = Sl // blk
    ntt = Tl // tt
    HP = nh * hd
    nhp = (HP + 127) // 128

    from contextlib import ExitStack

    with ExitStack() as ctx:
        wpool = ctx.enter_context(tc.tile_pool(name="weights", bufs=1))
        qtpool = ctx.enter_context(tc.tile_pool(name="qt", bufs=1))
        etpool = ctx.enter_context(tc.tile_pool(name="et", bufs=2))
        ktpool = ctx.enter_context(tc.tile_pool(name="kt", bufs=3))
        vpool = ctx.enter_context(tc.tile_pool(name="v", bufs=8))
        ppool = ctx.enter_context(tc.tile_pool(name="p", bufs=3))
        avpool = ctx.enter_context(tc.tile_pool(name="avacc", bufs=1))
        opool = ctx.enter_context(tc.tile_pool(name="outs", bufs=3))
        npool = ctx.enter_context(tc.tile_pool(name="norm", bufs=1))
        qo_ps = ctx.enter_context(tc.tile_pool(name="qo_ps", bufs=2, space="PSUM"))
        sc_ps = ctx.enter_context(tc.tile_pool(name="sc_ps", bufs=2, space="PSUM"))
        kv_ps = ctx.enter_context(tc.tile_pool(name="kv_ps", bufs=2, space="PSUM"))
        av_ps = ctx.enter_context(tc.tile_pool(name="av_ps", bufs=2, space="PSUM"))

        # ---- weights into SBUF ----
        wq_sb = [wpool.tile([128, HP], FR, tag=f"wq{dc}", name="wq_sb") for dc in range(ndc)]
        wk_sb = [wpool.tile([128, HP], FR, tag=f"wk{dc}", name="wk_sb") for dc in range(ndc)]
        wv_sb = [wpool.tile([128, HP], FR, tag=f"wv{dc}", name="wv_sb") for dc in range(ndc)]
        for dc in range(ndc):
            nc.sync.dma_start(out=wq_sb[dc], in_=wqT[dc * 128 : (dc + 1) * 128, :])
            nc.sync.dma_start(out=wk_sb[dc], in_=wkT[dc * 128 : (dc + 1) * 128, :])
            nc.sync.dma_start(out=wv_sb[dc], in_=wvT[dc * 128 : (dc + 1) * 128, :])
        wo_sb = [wpool.tile([hd, Dl], FR, tag=f"wo{h}", name="wo_sb") for h in range(nh)]
        for h in range(nh):
            nc.sync.dma_start(out=wo_sb[h], in_=woT[h * hd : (h + 1) * hd, :])
        bq_sb = wpool.tile([hd, nh], F32, tag="bq", name="bq_sb")
        bk_sb = wpool.tile([hd, nh], F32, tag="bk", name="bk_sb")
        for h in range(nh):
            nc.sync.dma_start(out=bq_sb[:, h : h + 1], in_=bqd[h])
            nc.sync.dma_start(out=bk_sb[:, h : h + 1], in_=bkd[h])
        bv_sb = wpool.tile([128, HP], F32, tag="bv", name="bv_sb")
        nc.sync.dma_start(out=bv_sb, in_=bvd)
        ones_sb = wpool.tile([128, 1], F32, tag="ones", name="ones_sb")
        nc.vector.memset(ones_sb, 1.0)

        # ---- phase 1: Q^T = (x @ Wq.T + bq)^T, packed heads on partitions ----
        qt_sb = {}  # (b, pass) -> [128, T] tile
        with tc.tile_pool(name="xt", bufs=1) as xpool:
            for b in range(nb):
                xts = []
                for dc in range(ndc):
                    xt = xpool.tile([128, Tl], FR, tag=f"xt{dc}", name="xt")
                    nc.sync.dma_start(
                        out=xt,
                        in_=xT[dc * 128 : (dc + 1) * 128, b * Tl : (b + 1) * Tl],
                    )
                    xts.append(xt)
                for h in range(nh):
                    qt = qtpool.tile([hd, Tl], FR, tag=f"qt{b}{h}", name="qt")
                    qt_sb[(b, h)] = qt
                    for nt in range(ntt):
                        ps = qo_ps.tile([128, tt], F32, tag="qo", name="q_psum")
                        for dc in range(ndc):
                            _mm(
                                nc,
                                ps[:hd, :],
                                wq_sb[dc][:, h * hd : (h + 1) * hd],
                                xts[dc][:, nt * tt : (nt + 1) * tt],
                                start=(dc == 0),
                                stop=(dc == ndc - 1),
                            )
                        nc.scalar.activation(
                            qt[:, nt * tt : (nt + 1) * tt],
                            ps[:hd, :],
                            AF.Identity,
                            bias=bq_sb[:, h : h + 1],
                        )

        # ---- phase 2+3 per batch ----
        for b in range(nb):
            avacc = {}
            for h in range(nh):
                avacc[h] = avpool.tile([hd + 1, Tl], F32, tag=f"av{h}", name="avacc")
            for ib in range(nblk):
                et = []
                for dc in range(ndc):
                    t = etpool.tile([128, blk], FR, tag=f"et{dc}", name="et_t")
                    nc.sync.dma_start(
                        out=t,
                        in_=eT[
                            dc * 128 : (dc + 1) * 128,
                            b * Sl + ib * blk : b * Sl + (ib + 1) * blk,
                        ],
                    )
                    et.append(t)
                for h in range(nh):
                    s = strides[h]
                    ncol = blk // s  # strided K/V rows in this block
                    # K^T_h for this block: [hd, ncol]
                    kt = ktpool.tile([hd, blk], FR, name="kt")
                    for c0 in range(0, ncol, tt):
                        cw = min(tt, ncol - c0)
                        kps = kv_ps.tile([128, tt], F32, tag="kv", name="kv_psum")
                        for dc in range(ndc):
                            _mm(
                                nc,
                                kps[:hd, :cw],
                                wk_sb[dc][:, h * hd : (h + 1) * hd],
                                et[dc][:, c0 * s : (c0 + cw) * s : s],
                                start=(dc == 0),
                                stop=(dc == ndc - 1),
                            )
                        nc.scalar.activation(
                            kt[:, c0 : c0 + cw],
                            kps[:hd, :cw],
                            AF.Identity,
                            bias=bk_sb[:, h : h + 1],
                        )
                    # V chunks + scores + exp + AV accumulation
                    nck = (ncol + 127) // 128
                    avp = [
                        av_ps.tile([hd + 1, tt], F32, tag="av_ps", name="av_psum") for _ in range(ntt)
                    ]
                    for ck in range(nck):
                        rw = min(128, ncol - ck * 128)
                        vt = vpool.tile([128, hd + 8], FR, tag="v", name="vt")
                        vps = kv_ps.tile([128, tt], F32, tag="kv", name="kv_psum")
                        for dc in range(ndc):
                            _mm(
                                nc,
                                vps[:rw, :hd],
                                et[dc][:, ck * 128 * s : (ck * 128 + rw) * s : s],
                                wv_sb[dc][:, h * hd : (h + 1) * hd],
                                start=(dc == 0),
                                stop=(dc == ndc - 1),
                            )
                        nc.vector.tensor_add(
                            vt[:rw, :hd], vps[:rw, :hd], bv_sb[:rw, h * hd : (h + 1) * hd]
                        )
                        nc.vector.tensor_copy(vt[:rw, hd : hd + 1], ones_sb[:rw])
                        # scores^T chunk: [rw, T], then P = exp(scores/8)
                        pt = ppool.tile([128, Tl], FR, tag="p", name="pt")
                        for nt in range(ntt):
                            sps = sc_ps.tile([128, tt], F32, tag="sc", name="sc_psum")
                            _mm(
                                nc,
                                sps[:rw, :],
                                kt[:, ck * 128 : ck * 128 + rw],
                                qt_sb[(b, h)][:, nt * tt : (nt + 1) * tt],
                                start=True,
                                stop=True,
                            )
                            nc.scalar.activation(
                                pt[:rw, nt * tt : (nt + 1) * tt],
                                sps[:rw, :],
                                AF.Exp,
                                scale=1.0 / float(np.sqrt(hd)),
                            )
                            _mm(
                                nc,
                                avp[nt],
                                vt[:rw, : hd + 1],
                                pt[:rw, nt * tt : (nt + 1) * tt],
                                start=(ck == 0),
                                stop=(ck == nck - 1),
                            )
                    for nt in range(ntt):
                        dst = avacc[h][:, nt * tt : (nt + 1) * tt]
                        if ib == 0:
                            nc.vector.tensor_copy(dst, avp[nt])
                        else:
                            nc.vector.tensor_add(dst, dst, avp[nt])

            # ---- phase 3: normalize + output projection ----
            ot_sb = {}
            for h in range(nh):
                r = npool.tile([1, Tl], F32, tag="recip", name="recip")
                nc.vector.reciprocal(r, avacc[h][hd : hd + 1, :])
                rb = npool.tile([hd, Tl], F32, tag="rb", name="rbcast")
                nc.gpsimd.partition_broadcast(rb, r)
                ot = npool.tile([hd, Tl], FR, tag=f"ot{h}", name="ot")
                nc.vector.tensor_mul(ot, avacc[h][:hd, :], rb)
                ot_sb[h] = ot
            for tc_i in range(Tl // 128):
                for nt in range(0, Dl, tt):
                    ops = qo_ps.tile([128, tt], F32, tag="qo", name="q_psum")
                    for h in range(nh):
                        _mm(
                            nc,
                            ops,
                            ot_sb[h][:, tc_i * 128 : (tc_i + 1) * 128],
                            wo_sb[h][:, nt : nt + tt],
                            start=(h == 0),
                            stop=(h == nh - 1),
                        )
                    ob = opool.tile([128, tt], F32, tag="ob", name="ob")
                    nc.vector.tensor_copy(ob, ops)
                    nc.sync.dma_start(
                        out=out[
                            b * Tl + tc_i * 128 : b * Tl + (tc_i + 1) * 128,
                            nt : nt + tt,
                        ],
                        in_=ob,
                    )


# ---------------------------------------------------------------------------
# Host-side sharding / gathering
# ---------------------------------------------------------------------------


def _core_map():
    """core -> (batches, heads)"""
    m = {}
    for c in range(N_CORES):
        g = c % 4
        bs = [0, 1] if c < 4 else [2, 3]
        hs = [4 * g + i for i in range(4)]
        m[c] = (bs, hs)
    return m


def shard_inputs(inputs, cfg):
    x = np.asarray(inputs["decoder_input"], np.float32)
    e = np.asarray(inputs["encoder_output"], np.float32)
    Wq = np.asarray(inputs["Wq"], np.float32)
    Wk = np.asarray(inputs["Wk"], np.float32)
    Wv = np.asarray(inputs["Wv"], np.float32)
    Wo = np.asarray(inputs["Wo"], np.float32)
    bq = np.asarray(inputs["bq"], np.float32)
    bk = np.asarray(inputs["bk"], np.float32)
    bv = np.asarray(inputs["bv"], np.float32)
    hd = cfg["hd"]
    nh = cfg["nh"]
    in_maps = []
    for c, (bs, hs) in _core_map().items():
        rows = np.concatenate([np.arange(h * hd, (h + 1) * hd) for h in hs])
        xT = np.ascontiguousarray(
            x[bs].reshape(len(bs) * cfg["T"], cfg["D"]).T.astype(BF16)
        )
        eTc = np.ascontiguousarray(
            e[bs].reshape(len(bs) * cfg["S"], cfg["D"]).T.astype(BF16)
        )
        in_maps.append(
            {
                "xT": xT,
                "eT": eTc,
                "wqT": np.ascontiguousarray(Wq[rows].T.astype(BF16)),
                "wkT": np.ascontiguousarray(Wk[rows].T.astype(BF16)),
                "wvT": np.ascontiguousarray(Wv[rows].T.astype(BF16)),
                "woT": np.ascontiguousarray(Wo[:, rows].T.astype(BF16)),
                "bq": np.ascontiguousarray(bq[rows].reshape(nh, hd, 1)),
                "bk": np.ascontiguousarray(bk[rows].reshape(nh, hd, 1)),
                "bvb": np.ascontiguousarray(
                    np.tile(bv[rows][None, :], (128, 1))
                ),
            }
        )
    return in_maps


def gather_output(results, bo, cfg):
    Tl, Dl = cfg["T"], cfg["D"]
    out = np.zeros((B, Tl, Dl), np.float32)
    for c, (bs, _hs) in _core_map().items():
        p = results[c]["partial"].reshape(len(bs), Tl, Dl)
        for i, b in enumerate(bs):
            out[b] += p[i]
    return out + np.asarray(bo, np.float32)[None, None, :]


_COMPILED = None


def _get_compiled():
    global _COMPILED
    if _COMPILED is None:
        _COMPILED = build_program(FULL_CFG)
    return _COMPILED


def run_on_cores(inputs, trace=False, **kw):
    nc = _get_compiled()
    in_maps = shard_inputs(inputs, FULL_CFG)
    res = bass_utils.run_bass_kernel_spmd(
        nc, in_maps, core_ids=list(range(N_CORES)), trace=trace, **kw
    )
    return res


def kernel(**inputs) -> np.ndarray:
    res = run_on_cores(inputs, trace=False)
    return gather_output(res.results, inputs["bo"], FULL_CFG)
